# revision 1
# baseline (speedup 1.0000x reference)
"""KV-cache attention Bass kernel for Trainium2, 8 NeuronCores.

Sharding: batch (4) x query-half (2) -> 8 cores. Each core projects Q for its
1024 query rows, projects the full new K/V for its batch (duplicated across the
core pair), and runs softmax(Q K^T / 8) V over the 8192-row concatenated cache.

Layout strategy (everything kept in matmul-native layouts, no on-device
transposes):
  - scores are computed TRANSPOSED: S^T[t, s] with lhsT = K^T tile, rhs = Q^T.
  - softmax over t (partition dim) uses exp with a constant shift (exact:
    softmax is shift-invariant) and gets the denominator from an extra
    all-ones column appended to V, so P^T @ [V | 1] yields both the
    numerator rows and the denominator in one PSUM accumulation.
  - P^T is exactly the stationary operand layout the PV matmul needs, so no
    transposes are ever required.
All matmuls run in float32r (full-rate fp32 mode, ~1.5e-4 rms error).
"""
import sys
import numpy as np

if "/opt/trn_rl_repo" not in sys.path:
    sys.path.insert(0, "/opt/trn_rl_repo")

import concourse.bacc as bacc
import concourse.mybir as mybir
from concourse.tile import TileContext
from concourse.bass_utils import run_bass_kernel_spmd

B, S_NEW, S_CACHE, D = 4, 2048, 6144, 1024
S_KV = S_CACHE + S_NEW            # 8192
SQ = S_NEW // 2                   # 1024 query rows per core
N_CORES = 8
P = 128
ET = D // P                       # 8 feature tiles
DT = D // P                       # 8 contraction tiles
CHUNK = 512                       # kv rows per chunk
N_CHUNKS = S_KV // CHUNK          # 16 (12 cached + 4 new)
N_CACHED_CHUNKS = S_CACHE // CHUNK
TT4 = CHUNK // P                  # 4 t-ptiles per chunk
SCALE = 0.125                     # 1/sqrt(64)
SHIFT = -16.0                     # constant softmax shift (exact)

F32 = mybir.dt.float32
F32R = mybir.dt.float32r

_cache = {}


def _build():
    nc = bacc.Bacc("TRN2", target_bir_lowering=False, debug=False,
                   num_devices=N_CORES)
    ht = nc.dram_tensor("ht", [P, DT * SQ], F32R, kind="ExternalInput")
    wq = nc.dram_tensor("wq", [P, DT * D], F32R, kind="ExternalInput")
    wk = nc.dram_tensor("wk", [P, DT * D], F32R, kind="ExternalInput")
    wv = nc.dram_tensor("wv", [P, DT * D], F32R, kind="ExternalInput")
    kcT = nc.dram_tensor("kcT", [P, ET, S_CACHE], F32R, kind="ExternalInput")
    vc = nc.dram_tensor("vc", [P, S_CACHE // P, D], F32R, kind="ExternalInput")
    bq = nc.dram_tensor("bq", [P, ET], F32, kind="ExternalInput")
    bk = nc.dram_tensor("bk", [P, ET], F32, kind="ExternalInput")
    bv = nc.dram_tensor("bv", [P, D], F32, kind="ExternalInput")
    ident = nc.dram_tensor("ident", [P, P], F32, kind="ExternalInput")
    out = nc.dram_tensor("out", [SQ, D], F32, kind="ExternalOutput")

    # Each core receives only ITS half of the new-token hidden states (the
    # host slices them), projects Q/K/V for that half, and the core pair
    # exchanges the new K/V halves with a 2-rank AllGather that overlaps
    # attention over the cached chunks.

    with TileContext(nc) as tc:
        with tc.tile_pool(name="big", bufs=1) as big, \
             tc.tile_pool(name="bias", bufs=1) as biasp, \
             tc.tile_pool(name="spsum", bufs=2, space="PSUM") as spsum, \
             tc.tile_pool(name="dpsum", bufs=2, space="PSUM") as dpsum, \
             tc.tile_pool(name="opsum", bufs=2, space="PSUM") as opsum, \
             tc.tile_pool(name="dram", bufs=1, space="DRAM") as dpool:

            nkT_h = dpool.tile([P, ET, SQ], F32R, name="nkT_h")
            nv_h = dpool.tile([P, SQ // P, D], F32R, name="nv_h")
            nkT_g = dpool.tile([2, P, ET, SQ], F32R, name="nkT_g")
            nv_g = dpool.tile([2, P, SQ // P, D], F32R, name="nv_g")

            qT_sb = big.tile([P, ET * SQ], F32R, name="qT_sb")
            bq_sb = biasp.tile([P, ET], F32, name="bq_sb")
            bk_sb = biasp.tile([P, ET], F32, name="bk_sb")
            bv_sb = biasp.tile([P, D], F32, name="bv_sb")
            sh_sb = biasp.tile([P, 1], F32, name="sh_sb")
            nc.vector.memset(sh_sb[:], SHIFT)
            ones_sb = biasp.tile([P, 2], F32, name="ones_sb")
            nc.vector.memset(ones_sb[:], 1.0)
            onesr_sb = biasp.tile([P, 2], F32R, name="onesr_sb")
            nc.vector.tensor_copy(onesr_sb[:], ones_sb[:])
            id_sb = biasp.tile([P, P], F32, name="id_sb")
            nc.sync.dma_start(out=id_sb[:], in_=ident[:])

            nc.sync.dma_start(out=bq_sb[:], in_=bq[:])
            nc.sync.dma_start(out=bk_sb[:], in_=bk[:])
            nc.sync.dma_start(out=bv_sb[:], in_=bv[:])

            early_cm = tc.tile_pool(name="early", bufs=1)
            earlyp = early_cm.__enter__()
            kt0_sb = earlyp.tile([P, ET, CHUNK], F32R, name="kt0_sb")
            v0_sb = earlyp.tile([P, TT4, D], F32R, name="v0_sb")

            self_a = tc.tile_pool(name="abig", bufs=1)
            abig = self_a.__enter__()
            wpool_cm = tc.tile_pool(name="wpool", bufs=2)
            wpool = wpool_cm.__enter__()
            stage_cm = tc.tile_pool(name="stage", bufs=3)
            stagep = stage_cm.__enter__()

            wq_sb = wpool.tile([P, DT * D], F32R, name="w_sb", tag="w")
            ht_sb = abig.tile([P, DT * SQ], F32R, name="ht_sb")
            for dt in range(DT):
                nc.sync.dma_start(out=wq_sb[:, dt * D:(dt + 1) * D],
                                  in_=wq[:, dt * D:(dt + 1) * D])
                nc.sync.dma_start(out=ht_sb[:, dt * SQ:(dt + 1) * SQ],
                                  in_=ht[:, dt * SQ:(dt + 1) * SQ])
            wk_sb = wpool.tile([P, DT * D], F32R, name="w_sb2", tag="w")
            nc.sync.dma_start(out=wk_sb[:], in_=wk[:])
            wv_sb = wpool.tile([P, DT * D], F32R, name="w_sb3", tag="w")
            nc.sync.dma_start(out=wv_sb[:], in_=wv[:])
            nc.sync.dma_start(out=kt0_sb[:], in_=kcT[:, :, 0:CHUNK])
            nc.sync.dma_start(out=v0_sb[:], in_=vc[:, 0:TT4, :])

            # ---- Phase A1: Q^T projection (queries are ht cols [0, SQ)) ----
            for et in range(ET):
                for sc in range(SQ // 512):
                    ps = spsum.tile([P, 512], F32, name="ps_q", tag="sp")
                    for dt in range(DT):
                        nc.tensor.matmul(
                            ps[:],
                            wq_sb[:, dt * D + et * P:dt * D + (et + 1) * P],
                            ht_sb[:, dt * SQ + sc * 512:dt * SQ + (sc + 1) * 512],
                            start=(dt == 0), stop=(dt == DT - 1))
                    nc.scalar.activation(
                        qT_sb[:, et * SQ + sc * 512:et * SQ + (sc + 1) * 512],
                        ps[:], mybir.ActivationFunctionType.Identity,
                        bias=bq_sb[:, et:et + 1])

            # ---- Phase A2: new K^T -> DRAM scratch ----
            for et in range(ET):
                for sc in range(SQ // 512):
                    ps = spsum.tile([P, 512], F32, name="ps_k", tag="sp")
                    for dt in range(DT):
                        nc.tensor.matmul(
                            ps[:],
                            wk_sb[:, dt * D + et * P:dt * D + (et + 1) * P],
                            ht_sb[:, dt * SQ + sc * 512:dt * SQ + (sc + 1) * 512],
                            start=(dt == 0), stop=(dt == DT - 1))
                    st = stagep.tile([P, 512], F32R, name="st_k", tag="stage")
                    nc.scalar.activation(
                        st[:], ps[:], mybir.ActivationFunctionType.Identity,
                        bias=bk_sb[:, et:et + 1])
                    nc.scalar.dma_start(
                        out=nkT_h[:, et, sc * 512:(sc + 1) * 512], in_=st[:])

            # ---- Phase A3: new V -> DRAM scratch ----
            for tt in range(SQ // P):
                for ec in range(D // 512):
                    ps = spsum.tile([P, 512], F32, name="ps_v", tag="sp")
                    for dt in range(DT):
                        nc.tensor.matmul(
                            ps[:],
                            ht_sb[:, dt * SQ + tt * P:dt * SQ + (tt + 1) * P],
                            wv_sb[:, dt * D + ec * 512:dt * D + (ec + 1) * 512],
                            start=(dt == 0), stop=(dt == DT - 1))
                    st = stagep.tile([P, 512], F32R, name="st_v", tag="stage")
                    nc.vector.tensor_add(st[:], ps[:], bv_sb[:, ec * 512:(ec + 1) * 512])
                    nc.scalar.dma_start(out=nv_h[:, tt, ec * 512:(ec + 1) * 512],
                                        in_=st[:])

            # ---- pair AllGather of the new K/V halves (overlaps attention
            # on the cached chunks) ----
            nc.gpsimd.collective_compute(
                "AllGather",
                mybir.AluOpType.bypass,
                replica_groups=[[0, 1], [2, 3], [4, 5], [6, 7]],
                ins=[nkT_h[:]],
                outs=[nkT_g[:]])
            nc.gpsimd.collective_compute(
                "AllGather",
                mybir.AluOpType.bypass,
                replica_groups=[[0, 1], [2, 3], [4, 5], [6, 7]],
                ins=[nv_h[:]],
                outs=[nv_g[:]])

            # free phase-A SBUF (ht, weights, staging) for phase-B pools
            stage_cm.__exit__(None, None, None)
            wpool_cm.__exit__(None, None, None)
            self_a.__exit__(None, None, None)

            kpool_cm = tc.tile_pool(name="kpool", bufs=2)
            kpool = kpool_cm.__enter__()
            vpool_cm = tc.tile_pool(name="vpool", bufs=2)
            vpool = vpool_cm.__enter__()
            ptpool_cm = tc.tile_pool(name="ptpool", bufs=2)
            ptpool = ptpool_cm.__enter__()
            fin_cm = tc.tile_pool(name="fin", bufs=2)
            finp = fin_cm.__enter__()
            obig_cm = tc.tile_pool(name="obig", bufs=1)
            obig = obig_cm.__enter__()
            out_acc = obig.tile([P, SQ // P, D], F32, name="out_acc")
            dn_acc = obig.tile([2, SQ], F32, name="dn_acc")

            # ---- Phase B: attention over 16 kv chunks ----
            for c in range(N_CHUNKS):
                if c == 0:
                    kt_sb, v_sb = kt0_sb, v0_sb
                elif True:
                    kt_sb = kpool.tile([P, ET, CHUNK], F32R, name="kt_sb")
                    v_sb = vpool.tile([P, TT4, D], F32R, name="v_sb")
                if c == 0:
                    pass
                elif c < N_CACHED_CHUNKS:
                    nc.sync.dma_start(out=kt_sb[:],
                                      in_=kcT[:, :, c * CHUNK:(c + 1) * CHUNK])
                    nc.sync.dma_start(out=v_sb[:],
                                      in_=vc[:, c * TT4:(c + 1) * TT4, :])
                else:
                    rank = (c - N_CACHED_CHUNKS) // 2
                    lc = (c - N_CACHED_CHUNKS) % 2
                    nc.sync.dma_start(
                        out=kt_sb[:],
                        in_=nkT_g[rank, :, :, lc * CHUNK:(lc + 1) * CHUNK])
                    nc.sync.dma_start(
                        out=v_sb[:],
                        in_=nv_g[rank, :, lc * TT4:(lc + 1) * TT4, :])

                for sb in range(SQ // 512):
                    pt = ptpool.tile([P, TT4, 512], F32R, name="pt")
                    for tt4 in range(TT4):
                        stp = spsum.tile([P, 512], F32, name="stp", tag="sp")
                        for et in range(ET):
                            nc.tensor.matmul(
                                stp[:],
                                kt_sb[:, et, tt4 * P:(tt4 + 1) * P],
                                qT_sb[:, et * SQ + sb * 512:et * SQ + (sb + 1) * 512],
                                start=(et == 0), stop=(et == ET - 1))
                        nc.scalar.activation(
                            pt[:, tt4, :], stp[:],
                            mybir.ActivationFunctionType.Exp,
                            bias=sh_sb[:], scale=SCALE)
                    dps = dpsum.tile([2, 512], F32, name="dps", tag="dps")
                    for tt4 in range(TT4):
                        nc.tensor.matmul(dps[:], onesr_sb[:],
                                         pt[:, tt4, :],
                                         start=(tt4 == 0), stop=(tt4 == TT4 - 1))
                    if c == 0:
                        nc.vector.tensor_copy(
                            dn_acc[0:2, sb * 512:(sb + 1) * 512], dps[0:2, :])
                    else:
                        nc.vector.tensor_add(
                            dn_acc[0:2, sb * 512:(sb + 1) * 512],
                            dn_acc[0:2, sb * 512:(sb + 1) * 512], dps[0:2, :])
                    for si in range(4):
                        si_g = sb * 4 + si
                        po = opsum.tile([P, D], F32, name="po")
                        for tt4 in range(TT4):
                            lhs = pt[:, tt4, si * P:(si + 1) * P]
                            st0 = (tt4 == 0)
                            sp1 = (tt4 == TT4 - 1)
                            nc.tensor.matmul(po[:, 0:512], lhs,
                                             v_sb[:, tt4, 0:512],
                                             start=st0, stop=sp1)
                            nc.tensor.matmul(po[:, 512:1024], lhs,
                                             v_sb[:, tt4, 512:1024],
                                             start=st0, stop=sp1)
                        if c == 0:
                            nc.vector.tensor_copy(out_acc[:, si_g, :], po[:])
                        else:
                            nc.vector.tensor_add(out_acc[:, si_g, :],
                                                 out_acc[:, si_g, :], po[:])
                        if c == N_CHUNKS - 1:
                            tps = dpsum.tile([P, 2], F32, name="tps", tag="dps")
                            nc.tensor.matmul(
                                tps[:], dn_acc[0:2, si_g * P:(si_g + 1) * P],
                                id_sb[0:2, 0:2], start=True, stop=True)
                            rec = finp.tile([P, 1], F32, name="rec")
                            nc.vector.reciprocal(rec[:], tps[:, 0:1])
                            ost = finp.tile([P, D], F32, name="ost")
                            nc.scalar.activation(
                                ost[:], out_acc[:, si_g, :D],
                                mybir.ActivationFunctionType.Copy,
                                scale=rec[:])
                            nc.sync.dma_start(
                                out=out[si_g * P:(si_g + 1) * P, :], in_=ost[:])

            obig_cm.__exit__(None, None, None)
            fin_cm.__exit__(None, None, None)
            ptpool_cm.__exit__(None, None, None)
            vpool_cm.__exit__(None, None, None)
            kpool_cm.__exit__(None, None, None)
            early_cm.__exit__(None, None, None)

    nc.compile()
    return nc


def _prep(hidden_states, cached_key, cached_value, Wq, bq, Wk, bk, Wv, bv):
    """Host-side resharding into SBUF-image layouts (pure reshapes/copies)."""
    def ptile_cols(a):  # [R, C] with R = n*128 -> [128, n*C] (partition-major)
        n = a.shape[0] // P
        return np.ascontiguousarray(
            a.reshape(n, P, a.shape[1]).transpose(1, 0, 2)).reshape(P, -1)

    w_h = {}
    for nm, W in (("wq", Wq), ("wk", Wk), ("wv", Wv)):
        w_h[nm] = ptile_cols(np.ascontiguousarray(W.T))          # [128, 8*1024]
    bq_h = np.ascontiguousarray(bq.reshape(ET, P).T)             # [128, 8]
    bk_h = np.ascontiguousarray(bk.reshape(ET, P).T)
    bv_h = np.ascontiguousarray(np.broadcast_to(bv, (P, D)))     # [128, 1024]
    id_h = np.eye(P, dtype=np.float32)

    in_maps = []
    for b in range(B):
        ht_full = ptile_cols(np.ascontiguousarray(hidden_states[b].T))  # [128, 8*2048]
        kcT_h = ptile_cols(np.ascontiguousarray(cached_key[b].T)) \
            .reshape(P, ET, S_CACHE)
        vc_h = np.ascontiguousarray(
            cached_value[b].reshape(S_CACHE // P, P, D).transpose(1, 0, 2))
        for h in range(2):
            ht_v = ht_full.reshape(P, DT, S_NEW)
            ht_c = np.ascontiguousarray(
                ht_v[:, :, h * SQ:(h + 1) * SQ]).reshape(P, DT * SQ)
            in_maps.append({
                "ht": ht_c, "kcT": kcT_h, "vc": vc_h,
                "wq": w_h["wq"], "wk": w_h["wk"], "wv": w_h["wv"],
                "bq": bq_h, "bk": bk_h, "bv": bv_h, "ident": id_h,
                "hsel": np.zeros((1, 1), np.float32),
            })
    return in_maps


def kernel(hidden_states, cached_key, cached_value, Wq, bq, Wk, bk, Wv, bv,
           _trace=False):
    if "nc" not in _cache:
        _cache["nc"] = _build()
    nc = _cache["nc"]
    in_maps = _prep(
        np.asarray(hidden_states, dtype=np.float32),
        np.asarray(cached_key, dtype=np.float32),
        np.asarray(cached_value, dtype=np.float32),
        np.asarray(Wq, dtype=np.float32), np.asarray(bq, dtype=np.float32),
        np.asarray(Wk, dtype=np.float32), np.asarray(bk, dtype=np.float32),
        np.asarray(Wv, dtype=np.float32), np.asarray(bv, dtype=np.float32))
    res = run_bass_kernel_spmd(nc, in_maps, list(range(N_CORES)), trace=_trace)
    _cache["last_result"] = res
    out = np.empty((B, S_NEW, D), np.float32)
    for b in range(B):
        for h in range(2):
            out[b, h * SQ:(h + 1) * SQ, :] = res.results[2 * b + h]["out"]
    return out



# revision 20
# speedup vs baseline: 1.1187x; 1.1187x over previous
"""KV-cache attention Bass kernel for Trainium2, 8 NeuronCores.

Sharding: batch (4) x query-half (2) -> 8 cores. Each core projects Q for its
1024 query rows, projects the new K/V for its half, exchanges the halves with a
2-rank AllGather, and runs softmax(Q K^T / 8) V over the 8192-row cache.

Layout/precision strategy:
  - All matmul operands in fp16 (full PE rate, ~1e-4 element error; psums stay
    fp32). Cached K^T / V are converted to fp16 on the host.
  - Scores computed transposed: S^T[t, s] with lhsT = K^T tile, rhs = Q^T, so
    the exp output P^T is exactly the stationary operand the PV matmul needs.
  - exp uses a constant shift (exact: softmax is shift-invariant); the
    denominator comes from ones-row matmuls accumulated in a pinned PSUM bank
    across all 16 kv chunks.
  - PV accumulates in PSUM across each 4-chunk group (16-matmul accumulation
    groups), flushed to an SBUF accumulator once per group, which removes
    almost all vector-engine traffic.
  - V bias is folded into the output (softmax rows sum to 1, so +bv commutes);
    Q/K biases ride the projection activations.
"""
import os
import sys
import numpy as np
import ml_dtypes

if "/opt/trn_rl_repo" not in sys.path:
    sys.path.insert(0, "/opt/trn_rl_repo")

DEBUG = os.environ.get("BASSDBG") == "1"

import concourse.bacc as bacc
import concourse.mybir as mybir
from concourse.tile import TileContext
from concourse.bass_utils import run_bass_kernel_spmd

B, S_NEW, S_CACHE, D = 4, 2048, 6144, 1024
S_KV = S_CACHE + S_NEW            # 8192
SQ = S_NEW // 2                   # 1024 query rows per core
N_CORES = 8
P = 128
ET = D // P                       # 8 feature tiles
DT = D // P                       # 8 contraction tiles
CHUNK = 512                       # kv rows per chunk
N_CHUNKS = S_KV // CHUNK          # 16 (12 cached + 4 new)
N_CACHED_CHUNKS = S_CACHE // CHUNK
TT4 = CHUNK // P                  # 4 t-ptiles per chunk
GROUP = 4                         # chunks per PV psum-accumulation group
N_GROUPS = N_CHUNKS // GROUP
SCALE = 0.125                     # 1/sqrt(64)
SHIFT = -16.0                     # constant softmax shift (exact)

F32 = mybir.dt.float32
F16 = mybir.dt.float16
BF16 = mybir.dt.bfloat16  # P/V operands: exp output needs the f32-like range

_cache = {}


def _build():
    nc = bacc.Bacc("TRN2", target_bir_lowering=False, debug=False,
                   num_devices=N_CORES)
    ht = nc.dram_tensor("ht", [P, DT, SQ], F16, kind="ExternalInput")
    wq = nc.dram_tensor("wq", [P, DT, D], F16, kind="ExternalInput")
    wk = nc.dram_tensor("wk", [P, DT, D], F16, kind="ExternalInput")
    wv = nc.dram_tensor("wv", [P, DT, D], F16, kind="ExternalInput")
    kcT = nc.dram_tensor("kcT", [P, ET, S_CACHE], F16, kind="ExternalInput")
    vc = nc.dram_tensor("vc", [P, S_CACHE // P, D], BF16, kind="ExternalInput")
    bq = nc.dram_tensor("bq", [P, ET], F32, kind="ExternalInput")
    bk = nc.dram_tensor("bk", [P, ET], F32, kind="ExternalInput")
    bvb = nc.dram_tensor("bvb", [P, D], F32, kind="ExternalInput")
    ident = nc.dram_tensor("ident", [P, P], F32, kind="ExternalInput")
    out = nc.dram_tensor("out", [SQ, D], F32, kind="ExternalOutput")
    if DEBUG:
        dbg_q = nc.dram_tensor("dbg_q", [P, ET, SQ], F16, kind="ExternalOutput")
        dbg_pt0 = nc.dram_tensor("dbg_pt0", [P, TT4, SQ], BF16,
                                 kind="ExternalOutput")
        dbg_pt15 = nc.dram_tensor("dbg_pt15", [P, TT4, SQ], BF16,
                                  kind="ExternalOutput")
        dbg_dn = nc.dram_tensor("dbg_dn", [2, SQ], F32, kind="ExternalOutput")
        dbg_acc = nc.dram_tensor("dbg_acc", [P, D], F32, kind="ExternalOutput")

    with TileContext(nc) as tc:
        with tc.tile_pool(name="persist", bufs=1) as persist, \
             tc.tile_pool(name="spsum", bufs=2, space="PSUM") as spsum, \
             tc.tile_pool(name="opsum", bufs=2, space="PSUM") as opsum, \
             tc.tile_pool(name="dnpsum", bufs=1, space="PSUM") as dnp, \
             tc.tile_pool(name="dram", bufs=1, space="DRAM") as dpool:

            nkT_h = dpool.tile([P, ET, SQ], F16, name="nkT_h")
            nv_h = dpool.tile([P, SQ // P, D], BF16, name="nv_h")
            nkT_g = dpool.tile([2, P, ET, SQ], F16, name="nkT_g")
            nv_g = dpool.tile([2, P, SQ // P, D], BF16, name="nv_g")

            qT = persist.tile([P, ET, SQ], F16, name="qT")
            bq_sb = persist.tile([P, ET], F32, name="bq_sb")
            bk_sb = persist.tile([P, ET], F32, name="bk_sb")
            bv_sb = persist.tile([P, D], F32, name="bv_sb")
            sh_sb = persist.tile([P, 1], F32, name="sh_sb")
            nc.vector.memset(sh_sb[:], SHIFT)
            ones_f = persist.tile([P, 2], F32, name="ones_f")
            nc.vector.memset(ones_f[:], 1.0)
            ones16 = persist.tile([P, 2], BF16, name="ones16")
            nc.vector.tensor_copy(ones16[:], ones_f[:])
            id_sb = persist.tile([P, P], F32, name="id_sb")
            nc.sync.dma_start(out=id_sb[:], in_=ident[:])
            nc.sync.dma_start(out=bq_sb[:], in_=bq[:])
            nc.sync.dma_start(out=bk_sb[:], in_=bk[:])
            nc.sync.dma_start(out=bv_sb[:], in_=bvb[:])

            out_acc = persist.tile([P, SQ // P, D], F32, name="out_acc")

            # chunk streaming pools (persist across both phases)
            kpool_cm = tc.tile_pool(name="kpool", bufs=3)
            kpool = kpool_cm.__enter__()
            vpool_cm = tc.tile_pool(name="vpool", bufs=6)
            vpool = vpool_cm.__enter__()

            kt_tiles = [None] * N_CHUNKS
            v_tiles = [None] * N_CHUNKS
            pt_tiles = [None] * N_CHUNKS

            def load_kt(c):
                t = kpool.tile([P, ET, CHUNK], F16, name="kt", tag="kt")
                if c < N_CACHED_CHUNKS:
                    nc.sync.dma_start(
                        out=t[:], in_=kcT[:, :, c * CHUNK:(c + 1) * CHUNK])
                else:
                    rank = (c - N_CACHED_CHUNKS) // 2
                    lc = (c - N_CACHED_CHUNKS) % 2
                    nc.sync.dma_start(
                        out=t[:],
                        in_=nkT_g[rank, :, :, lc * CHUNK:(lc + 1) * CHUNK])
                kt_tiles[c] = t

            def load_v(c):
                t = vpool.tile([P, TT4, D], BF16, name="v", tag="v")
                if c < N_CACHED_CHUNKS:
                    nc.sync.dma_start(
                        out=t[:], in_=vc[:, c * TT4:(c + 1) * TT4, :])
                else:
                    rank = (c - N_CACHED_CHUNKS) // 2
                    lc = (c - N_CACHED_CHUNKS) % 2
                    nc.sync.dma_start(
                        out=t[:],
                        in_=nv_g[rank, :, lc * TT4:(lc + 1) * TT4, :])
                v_tiles[c] = t

            # ---- Phase A: fp16 projections ----
            pha_cm = tc.tile_pool(name="pha", bufs=1)
            pha = pha_cm.__enter__()
            stage_cm = tc.tile_pool(name="stage", bufs=4)
            stagep = stage_cm.__enter__()

            wq_sb = pha.tile([P, DT, D], F16, name="wq_sb")
            ht_sb = pha.tile([P, DT, SQ], F16, name="ht_sb")
            for dt in range(DT):
                nc.scalar.dma_start(out=wq_sb[:, dt, :], in_=wq[:, dt, :])
                nc.sync.dma_start(out=ht_sb[:, dt, :], in_=ht[:, dt, :])
            wk_sb = pha.tile([P, DT, D], F16, name="wk_sb")
            nc.gpsimd.dma_start(out=wk_sb[:], in_=wk[:])
            wv_sb = pha.tile([P, DT, D], F16, name="wv_sb")
            nc.gpsimd.dma_start(out=wv_sb[:], in_=wv[:])
            load_kt(0)
            load_kt(1)
            load_v(0)

            # A1: Q^T  (psum [e 128, s 512] <- sum_dt wq^T ht)
            for et in range(ET):
                for sc in range(SQ // 512):
                    ps = spsum.tile([P, 512], F32, name="ps_q", tag="sp")
                    for dt in range(DT):
                        nc.tensor.matmul(
                            ps[:],
                            wq_sb[:, dt, et * P:(et + 1) * P],
                            ht_sb[:, dt, sc * 512:(sc + 1) * 512],
                            start=(dt == 0), stop=(dt == DT - 1))
                    nc.scalar.activation(
                        qT[:, et, sc * 512:(sc + 1) * 512],
                        ps[:], mybir.ActivationFunctionType.Identity,
                        bias=bq_sb[:, et:et + 1])

            # A2: new K^T -> DRAM scratch
            for et in range(ET):
                for sc in range(SQ // 512):
                    ps = spsum.tile([P, 512], F32, name="ps_k", tag="sp")
                    for dt in range(DT):
                        nc.tensor.matmul(
                            ps[:],
                            wk_sb[:, dt, et * P:(et + 1) * P],
                            ht_sb[:, dt, sc * 512:(sc + 1) * 512],
                            start=(dt == 0), stop=(dt == DT - 1))
                    st = stagep.tile([P, 512], F16, name="st_k", tag="stage")
                    nc.scalar.activation(
                        st[:], ps[:], mybir.ActivationFunctionType.Identity,
                        bias=bk_sb[:, et:et + 1])
                    nc.scalar.dma_start(
                        out=nkT_h[:, et, sc * 512:(sc + 1) * 512], in_=st[:])

            # A3: new V -> DRAM scratch (bias folded into final output)
            for tt in range(SQ // P):
                for ec in range(D // 512):
                    ps = spsum.tile([P, 512], F32, name="ps_v", tag="sp")
                    for dt in range(DT):
                        nc.tensor.matmul(
                            ps[:],
                            ht_sb[:, dt, tt * P:(tt + 1) * P],
                            wv_sb[:, dt, ec * 512:(ec + 1) * 512],
                            start=(dt == 0), stop=(dt == DT - 1))
                    st = stagep.tile([P, 512], BF16, name="st_v", tag="stage")
                    nc.scalar.activation(
                        st[:], ps[:], mybir.ActivationFunctionType.Copy)
                    nc.scalar.dma_start(
                        out=nv_h[:, tt, ec * 512:(ec + 1) * 512], in_=st[:])

            # pair AllGather of the new K/V halves (overlaps cached attention)
            nc.gpsimd.collective_compute(
                "AllGather", mybir.AluOpType.bypass,
                replica_groups=[[0, 1], [2, 3], [4, 5], [6, 7]],
                ins=[nkT_h[:]], outs=[nkT_g[:]])
            nc.gpsimd.collective_compute(
                "AllGather", mybir.AluOpType.bypass,
                replica_groups=[[0, 1], [2, 3], [4, 5], [6, 7]],
                ins=[nv_h[:]], outs=[nv_g[:]])

            stage_cm.__exit__(None, None, None)
            pha_cm.__exit__(None, None, None)

            ppool_cm = tc.tile_pool(name="ppool", bufs=2 * GROUP)
            ppool = ppool_cm.__enter__()

            # ---- Phase B: attention, 4 groups of 4 chunks ----
            dn_ps = [dnp.tile([2, 512], F32, name=f"dn{sb}") for sb in range(2)]

            def s_pass(c):
                kt = kt_tiles[c]
                pt = ppool.tile([P, TT4, SQ], BF16, name="pt", tag="pt")
                pt_tiles[c] = pt
                for sb in range(SQ // 512):
                    for tt4 in range(TT4):
                        ps = spsum.tile([P, 512], F32, name="ps_s", tag="sp")
                        for et in range(ET):
                            nc.tensor.matmul(
                                ps[:],
                                kt[:, et, tt4 * P:(tt4 + 1) * P],
                                qT[:, et, sb * 512:(sb + 1) * 512],
                                start=(et == 0), stop=(et == ET - 1))
                        nc.scalar.activation(
                            pt[:, tt4, sb * 512:(sb + 1) * 512], ps[:],
                            mybir.ActivationFunctionType.Exp,
                            bias=sh_sb[:], scale=SCALE)
                    for tt4 in range(TT4):
                        nc.tensor.matmul(
                            dn_ps[sb][:], ones16[:, 0:2],
                            pt[:, tt4, sb * 512:(sb + 1) * 512],
                            start=(c == 0 and tt4 == 0),
                            stop=(c == N_CHUNKS - 1 and tt4 == TT4 - 1))
                if DEBUG and c == 0:
                    nc.scalar.dma_start(out=dbg_pt0[:], in_=pt[:])
                if DEBUG and c == N_CHUNKS - 1:
                    nc.scalar.dma_start(out=dbg_pt15[:], in_=pt[:])

            fin_cm = tc.tile_pool(name="fin", bufs=2)
            finp = fin_cm.__enter__()
            dnZ = persist.tile([2, SQ], F32, name="dnZ")
            rec_sb = persist.tile([P, SQ // P], F32, name="rec_sb")

            def finalize_si(si):
                ost = finp.tile([P, D], F32, name="ost")
                nc.scalar.activation(
                    ost[:], out_acc[:, si, :],
                    mybir.ActivationFunctionType.Copy,
                    scale=rec_sb[:, si:si + 1])
                nc.vector.tensor_add(ost[:], ost[:], bv_sb[:])
                nc.scalar.dma_start(out=out[si * P:(si + 1) * P, :], in_=ost[:])

            def pv_pass(g):
                c0 = g * GROUP
                last = g == N_GROUPS - 1
                if last:
                    # denominators stopped accumulating at the end of s_pass
                    # (c == 15): prep reciprocals while PV matmuls run
                    for sb in range(2):
                        nc.vector.tensor_copy(
                            dnZ[0:2, sb * 512:(sb + 1) * 512], dn_ps[sb][:])
                    if DEBUG:
                        nc.scalar.dma_start(out=dbg_q[:], in_=qT[:])
                        nc.scalar.dma_start(out=dbg_dn[:], in_=dnZ[:])
                    for si in range(SQ // P):
                        tps = spsum.tile([P, 2], F32, name="tps", tag="sp")
                        nc.tensor.matmul(tps[:], dnZ[0:2, si * P:(si + 1) * P],
                                         id_sb[0:2, 0:2], start=True, stop=True)
                        nc.vector.reciprocal(rec_sb[:, si:si + 1], tps[:, 0:1])
                for si in range(SQ // P):
                    for dh in range(D // 512):
                        po = opsum.tile([P, 512], F32, name="po", tag="po")
                        for ci in range(GROUP):
                            pt = pt_tiles[c0 + ci]
                            v = v_tiles[c0 + ci]
                            for tt4 in range(TT4):
                                nc.tensor.matmul(
                                    po[:],
                                    pt[:, tt4, si * P:(si + 1) * P],
                                    v[:, tt4, dh * 512:(dh + 1) * 512],
                                    start=(ci == 0 and tt4 == 0),
                                    stop=(ci == GROUP - 1 and tt4 == TT4 - 1))
                        if g == 0:
                            nc.vector.tensor_copy(
                                out_acc[:, si, dh * 512:(dh + 1) * 512], po[:])
                        else:
                            nc.vector.tensor_add(
                                out_acc[:, si, dh * 512:(dh + 1) * 512],
                                out_acc[:, si, dh * 512:(dh + 1) * 512], po[:])
                    if last:
                        if DEBUG and si == 0:
                            nc.scalar.dma_start(out=dbg_acc[:],
                                                in_=out_acc[:, 0, :])
                        finalize_si(si)

            for g in range(N_GROUPS):
                for ci in range(GROUP):
                    c = g * GROUP + ci
                    if c + 2 < N_CHUNKS:
                        load_kt(c + 2)
                    if c + 1 < N_CHUNKS:
                        load_v(c + 1)
                    s_pass(c)
                pv_pass(g)

            fin_cm.__exit__(None, None, None)
            ppool_cm.__exit__(None, None, None)
            vpool_cm.__exit__(None, None, None)
            kpool_cm.__exit__(None, None, None)

    nc.compile()
    return nc


def _prep(hidden_states, cached_key, cached_value, Wq, bq, Wk, bk, Wv, bv):
    """Host-side resharding into SBUF-image layouts (reshapes + fp16 casts)."""
    f16 = np.float16

    def ptile_cols(a):  # [R, C] with R = n*128 -> [128, n, C] (partition-major)
        n = a.shape[0] // P
        return np.ascontiguousarray(a.reshape(n, P, a.shape[1]).transpose(1, 0, 2))

    w_h = {}
    for nm, W in (("wq", Wq), ("wk", Wk), ("wv", Wv)):
        w_h[nm] = ptile_cols(np.ascontiguousarray(W.T)).astype(f16)  # [128,8,1024]
    bq_h = np.ascontiguousarray(bq.reshape(ET, P).T)                 # [128, 8]
    bk_h = np.ascontiguousarray(bk.reshape(ET, P).T)
    bv_h = np.ascontiguousarray(np.broadcast_to(bv, (P, D))).astype(np.float32)
    id_h = np.eye(P, dtype=np.float32)

    in_maps = []
    for b in range(B):
        ht_full = ptile_cols(np.ascontiguousarray(hidden_states[b].T))  # [128,8,2048]
        kcT_h = ptile_cols(np.ascontiguousarray(cached_key[b].T)).astype(f16)
        vc_h = np.ascontiguousarray(
            cached_value[b].reshape(S_CACHE // P, P, D).transpose(1, 0, 2)
        ).astype(ml_dtypes.bfloat16)
        for h in range(2):
            ht_c = np.ascontiguousarray(
                ht_full[:, :, h * SQ:(h + 1) * SQ]).astype(f16)
            in_maps.append({
                "ht": ht_c, "kcT": kcT_h, "vc": vc_h,
                "wq": w_h["wq"], "wk": w_h["wk"], "wv": w_h["wv"],
                "bq": bq_h, "bk": bk_h, "bvb": bv_h, "ident": id_h,
            })
    return in_maps


def kernel(hidden_states, cached_key, cached_value, Wq, bq, Wk, bk, Wv, bv,
           _trace=False):
    if "nc" not in _cache:
        _cache["nc"] = _build()
    nc = _cache["nc"]
    in_maps = _prep(
        np.asarray(hidden_states, dtype=np.float32),
        np.asarray(cached_key, dtype=np.float32),
        np.asarray(cached_value, dtype=np.float32),
        np.asarray(Wq, dtype=np.float32), np.asarray(bq, dtype=np.float32),
        np.asarray(Wk, dtype=np.float32), np.asarray(bk, dtype=np.float32),
        np.asarray(Wv, dtype=np.float32), np.asarray(bv, dtype=np.float32))
    res = run_bass_kernel_spmd(nc, in_maps, list(range(N_CORES)), trace=_trace)
    _cache["last_result"] = res
    out = np.empty((B, S_NEW, D), np.float32)
    for b in range(B):
        for h in range(2):
            out[b, h * SQ:(h + 1) * SQ, :] = res.results[2 * b + h]["out"]
    return out
